# revision 2
# baseline (speedup 1.0000x reference)
"""Multi-head attention (QKV proj + softmax(QK^T)V) on 8 TRN2 NeuronCores.

Sharding: 8 cores = 4 batches x 2 head-groups (6 heads each). Pure data
parallel - no collectives. Host pre-transposes shards so every on-device
matmul streams with zero on-chip transposes.

The kernel is PE-bound: S (65us) + AV (104us) + projections (52us) =
~220us of tensor-engine work vs 200us of ScalarE exp. Everything is
scheduled so the in-order PE never waits:
  - input DMAs kt-halved and interleaved across BOTH hwdge queues in
    deadline order, so each tensor's halves land concurrently and the
    head projections can start on the first half
  - PE warmup matmuls at t=0 beat the DVFS ramp (full clock needs ~3us
    of continuous execution) so the head projections run at speed
  - per-core pipeline (transposed layouts, d-on-partitions):
      wqT = WqT.T @ qT + bq   [384,2048] (pair p -> head 2p rows 0:64, 2p+1 rows 64:128)
      wv  = vT.T @ WvT + bv   (stored per seq-tile, ones column rides col 64)
      S^T = wkT.T @ wqT per head pair via PE quadrants (tile_position)
      exp on ScalarE (no max subtraction; scores <~70, fp32-safe)
      [out.T; rowsum] = [wv | 1].T @ P^T  (denominator rides the AV matmul)
  - projection units are inserted on a deadline schedule derived from a
    DMA bandwidth model; first-qchunk AV pairs are GATED (deferred up to
    the 24-slot p-ring depth) so late v chunks never stall the PE
  - normalize: recip on DVE via [128,8] bounce (partition-serial otherwise),
    partition-broadcast on GpSimd mid-stream / K=1 matmul at the tail,
    final flush runs at high priority on the then-idle scalar queue.
"""

import sys

if "/opt/trn_rl_repo" not in sys.path:
    sys.path.insert(0, "/opt/trn_rl_repo")

import numpy as np

_TILEPOS = True

_BS, _SEQ, _DM = 4, 2048, 768
_NH, _DH = 12, 64
_GSZ = _DM // 2  # 384 dims per head-group
_NCORES = 8

_KT = _DM // 128   # 6 contraction tiles
_ST = _SEQ // 128  # 16 seq tiles (key positions)
_QC = _SEQ // 512  # 4 query chunks
_NP = _GSZ // 128  # 3 head pairs

_compiled = None


def _insert_schedule():
    """global-iter -> list of insert items. iter = 64*p + 16*qch + kt.

    Items: ("V", st, 0) one V-projection unit (6 matmuls of 384 cols);
           ("qk", which, m, nch, c0, w) q/k unit (6 matmuls of w cols).
    Placement follows a DMA model (two hwdge queues, deadline-ordered
    kt-halved transfers, ~360-490 KB/us aggregate) so each unit's data
    is present when the in-order PE reaches it.
    """
    ins = {}

    def add(it, item):
        ins.setdefault(it, []).append(item)

    # pair-0 k/q: k(0,nch) must precede S kt=4nch (emitted at gi 4nch-2);
    # q(0,c) before S of qchunk c (emitted at gi 16c-2).
    add(1, ("qk", "k", 0, 1, 0, 512))
    add(5, ("qk", "k", 0, 2, 0, 512))
    add(9, ("qk", "k", 0, 3, 0, 512))
    add(13, ("qk", "q", 0, 1, 0, 512))
    add(29, ("qk", "q", 0, 2, 0, 512))
    add(45, ("qk", "q", 0, 3, 0, 512))
    # V units: gated AVs (see _AV_GATE) tolerate late placement; spread
    # per the v-chunk DMA arrivals.
    for st in range(4):
        add(9 + st, ("V", st, 0))
    for st in range(4, 8):
        add(13 + (st - 4), ("V", st, 0))
    for st in range(8, 12):
        add(17 + (st - 8), ("V", st, 0))
    for st in range(12, 16):
        add(21 + (st - 12), ("V", st, 0))
    # pair 1 (deadlines: k(1,n) by 61+4n, q(1,c) by 61+16c)
    for nch in range(4):
        add(46 + 3 * nch, ("qk", "k", 1, nch, 0, 512))
    add(59, ("qk", "q", 1, 0, 0, 512))
    for qch, base in ((1, 74), (2, 90), (3, 105)):
        add(base, ("qk", "q", 1, qch, 0, 512))
    # pair 2 (deadlines: k(2,n) by 125+4n, q(2,c) by 125+16c)
    for nch in range(4):
        add(110 + 3 * nch, ("qk", "k", 2, nch, 0, 512))
    add(122, ("qk", "q", 2, 0, 0, 512))
    for qch, base in ((1, 138), (2, 154), (3, 170)):
        add(base, ("qk", "q", 2, qch, 0, 512))

    # safety: every unit placed before its consumer
    for it, items in ins.items():
        for item in items:
            if item[0] == "V":
                pass  # V deadline enforced via _AV_GATE
            else:
                _, which, m, nch, c0, w = item
                if which == "k":
                    dl = 64 * m + 4 * nch - 2
                else:
                    dl = 64 * m + 16 * nch - 2
                assert it <= max(dl, 1) or m == 0 and nch <= 1, (it, item)
    return ins


def _av_gates(inserts):
    """Earliest gi at which AV pair k (global 0..191) may be emitted.

    qchunk 0 (pair 0): gated one iter after its V insert so a late v DMA
    stalls only the latency-tolerant AV stream, not S/exp. Later
    qchunks: classic 2-deferral (boundary S-pairs run ahead of the
    accumulator-eviction wait).
    """
    v_gi = {}
    for gi, items in inserts.items():
        for item in items:
            if item[0] == "V":
                v_gi[item[1]] = gi
    gates = []
    for k in range(_NP * _QC * _ST):
        qi, kt = divmod(k, _ST)
        if qi == 0:
            gates.append(v_gi[kt] + 1)
        else:
            gates.append(16 * qi + max(kt, 2))
    return gates


def _build():
    import concourse.bass as bass  # noqa: F401
    import concourse.mybir as mybir
    import concourse.tile as tile
    from concourse import bacc

    f32 = mybir.dt.float32
    bf16 = mybir.dt.bfloat16
    AF = mybir.ActivationFunctionType

    nc = bacc.Bacc("TRN2", target_bir_lowering=False, debug=False)

    # q/k/v packed [p, nch, kt, c]: one nch slice = 128 descriptors x 6KB
    q_pk = nc.dram_tensor("q_pk", [128, 4, _KT, 512], bf16, kind="ExternalInput")
    k_pk = nc.dram_tensor("k_pk", [128, 4, _KT, 512], bf16, kind="ExternalInput")
    v_pk = nc.dram_tensor("v_pk", [128, 4, _KT, 512], bf16, kind="ExternalInput")
    # Wq/Wk packed [p, m, kt, c]: one m slab = 128 x 1.5KB
    Wq_pk = nc.dram_tensor("Wq_pk", [128, _NP, _KT, 128], bf16, kind="ExternalInput")
    Wk_pk = nc.dram_tensor("Wk_pk", [128, _NP, _KT, 128], bf16, kind="ExternalInput")
    Wv_pk = nc.dram_tensor("Wv_pk", [128, _KT, _GSZ], bf16, kind="ExternalInput")
    # biases packed together: cols 0:3 bqT, 3:6 bkT, 6:390 bv (row-replicated)
    b_all = nc.dram_tensor("b_all", [128, 6 + _GSZ], f32, kind="ExternalInput")
    outT = nc.dram_tensor("outT", [_GSZ, _SEQ], f32, kind="ExternalOutput")

    INSERTS = _insert_schedule()
    AV_GATE = _av_gates(INSERTS)
    N_GI = _NP * _QC * _ST

    with tile.TileContext(nc) as tc:
        with (
            tc.tile_pool(name="persist", bufs=1) as persist,
            tc.tile_pool(name="qkv", bufs=1) as qkv_pool,
            tc.tile_pool(name="w", bufs=1) as w_pool,
            tc.tile_pool(name="psum", bufs=2, space="PSUM") as psum,
            tc.tile_pool(name="att", bufs=4) as att_pool,
        ):
            # ---- persistent SBUF ----
            wqT_sb = [persist.tile([128, _SEQ], bf16, tag=f"wqT{p}", name=f"wqT{p}")
                      for p in range(_NP)]
            wkT_sb = [persist.tile([128, _SEQ], bf16, tag=f"wkT{p}", name=f"wkT{p}")
                      for p in range(_NP)]
            # per seq-tile, per head: [64 wv dims | ones | pad]
            wv_sb = persist.tile([128, _ST, 6, 66], bf16, tag="wv")
            for st in range(_ST):
                nc.vector.memset(wv_sb[:, st, :, 64:65], 1.0)
            ones_sb = persist.tile([1, 64], f32, tag="ones")
            nc.vector.memset(ones_sb[:, :], 1.0)
            ones16 = persist.tile([1, 64], bf16, tag="ones16")
            nc.vector.memset(ones16[:, :], 1.0)
            warm_sb = persist.tile([1, 512], bf16, tag="warm")
            nc.vector.memset(warm_sb[:, :], 0.0)

            q_all = qkv_pool.tile([128, 4, _KT, 512], bf16, tag="qa", name="q_all")
            k_all = qkv_pool.tile([128, 4, _KT, 512], bf16, tag="ka", name="k_all")
            v_all = qkv_pool.tile([128, 4, _KT, 512], bf16, tag="va", name="v_all")
            wq_all = w_pool.tile([128, _NP, _KT, 128], bf16, tag="wqa", name="wq_all")
            wk_all = w_pool.tile([128, _NP, _KT, 128], bf16, tag="wka", name="wk_all")
            wv_all = w_pool.tile([128, _KT, _GSZ], bf16, tag="wva", name="wv_all")
            b_sb = persist.tile([128, 6 + _GSZ], f32, tag="b")

            # ---- input DMAs: kt-halved, interleaved across both queues in
            # deadline order. Halves land concurrently so each tensor
            # completes in ~half the serial time, and the head projection's
            # per-kt matmuls start on the first half.
            kh = _KT // 2

            def dma_pair(dst, src):
                nc.sync.dma_start(dst[:, 0:kh], src[:, 0:kh])
                nc.scalar.dma_start(dst[:, kh:_KT], src[:, kh:_KT])

            nc.sync.dma_start(wk_all[:, 0], Wk_pk[:, 0])
            nc.scalar.dma_start(b_sb[:, :], b_all[:, :])
            nc.scalar.dma_start(wq_all[:, 0], Wq_pk[:, 0])
            dma_pair(k_all[:, 0], k_pk[:, 0])
            dma_pair(q_all[:, 0], q_pk[:, 0])
            dma_pair(k_all[:, 1], k_pk[:, 1])
            dma_pair(k_all[:, 2], k_pk[:, 2])
            dma_pair(k_all[:, 3], k_pk[:, 3])
            nc.sync.dma_start(wv_all[:, 0:kh], Wv_pk[:, 0:kh])
            nc.scalar.dma_start(wv_all[:, kh:_KT], Wv_pk[:, kh:_KT])
            dma_pair(q_all[:, 1], q_pk[:, 1])
            dma_pair(v_all[:, 0], v_pk[:, 0])
            dma_pair(v_all[:, 1], v_pk[:, 1])
            dma_pair(v_all[:, 2], v_pk[:, 2])
            dma_pair(v_all[:, 3], v_pk[:, 3])
            dma_pair(q_all[:, 2], q_pk[:, 2])
            dma_pair(q_all[:, 3], q_pk[:, 3])
            nc.sync.dma_start(wk_all[:, 1], Wk_pk[:, 1])
            nc.scalar.dma_start(wq_all[:, 1], Wq_pk[:, 1])
            nc.sync.dma_start(wk_all[:, 2], Wk_pk[:, 2])
            nc.scalar.dma_start(wq_all[:, 2], Wq_pk[:, 2])

            # ---- PE warmup: the tensor engine needs ~3us of continuous
            # execution to reach full clock. Dummy matmuls (zeros moving)
            # fill the DMA wait so the head projections run at speed.
            for wi in range(5):
                wps = psum.tile([64, 512], f32, tag="ins", name="warm_ps",
                                padded_shape=[128, 512], bufs=2)
                nc.tensor.matmul(wps[0:64, :], ones16[0:1, 0:64],
                                 warm_sb[0:1, :], start=True, stop=True)

            # ---- projection unit emitters ----
            def emit_v_unit(st, pr, npair=1):
                nch, cc = st // 4, (st % 4) * 128
                d0, dn = 128 * pr, 128 * npair
                nh = 2 * npair
                psv = psum.tile([128, dn], f32, tag="ins", name="psv",
                                padded_shape=[128, 512], bufs=2)
                for t in range(_KT):
                    nc.tensor.matmul(
                        psv[:, :], v_all[:, nch, t, cc:cc + 128],
                        wv_all[:, t, d0:d0 + dn],
                        start=(t == 0), stop=(t == _KT - 1),
                    )
                nc.vector.tensor_add(
                    wv_sb[:, st, 2 * pr:2 * pr + nh, 0:64],
                    psv[:, :].rearrange("p (h d) -> p h d", h=nh),
                    b_sb[:, 6 + d0:6 + d0 + dn].rearrange(
                        "p (h d) -> p h d", h=nh),
                )

            def emit_qk_part(which, m, nch, c0, w):
                ps = psum.tile([128, w], f32, tag="ins", name="psqk",
                               padded_shape=[128, 512], bufs=2)
                x_all, w_all, dst, boff = (
                    (q_all, wq_all, wqT_sb, 0) if which == "q"
                    else (k_all, wk_all, wkT_sb, 3)
                )
                for t in range(_KT):
                    nc.tensor.matmul(
                        ps[:, :], w_all[:, m, t, :],
                        x_all[:, nch, t, c0:c0 + w],
                        start=(t == 0), stop=(t == _KT - 1),
                    )
                nc.vector.tensor_scalar_add(
                    dst[m][:, nch * 512 + c0:nch * 512 + c0 + w],
                    ps[:, :], b_sb[:, boff + m:boff + m + 1])

            def emit_insert(item):
                if item[0] == "V":
                    emit_v_unit(item[1], item[2], npair=3)
                else:
                    _, which, m, nch, c0, w = item
                    emit_qk_part(which, m, nch, c0, w)

            # ---- head: pair-0 qchunk-0 q/k projections ----
            # SPINE priority offset: the attention spine (S-pairs, exps, and
            # the head k00/q00 that gate exp#1) is emitted with a large
            # priority offset so the Tile list-scheduler never displaces
            # ready spine work with projection fillers whose DMA data may be
            # late on real hardware.
            with tc.high_priority():
                emit_qk_part("k", 0, 0, 0, 512)
                emit_qk_part("q", 0, 0, 0, 512)

            # ---- attention stream ----
            # deferred normalize state: (av_sb, recip, hA, hB, qsl) of the
            # previous qchunk; its broadcast + muls + out DMAs are emitted a
            # few iterations later so the in-order PE stream never stalls on
            # the reciprocal round-trip at a boundary.
            pending = []
            flush_gate = [None]  # gi at which pending may flush

            def flush_pending(final=False):
                av_sb, recip, fA, fB, fqsl = pending.pop()
                o_sb = att_pool.tile([64, 1024], f32, tag="o", name="o_sb",
                                     bufs=2)
                if not final:
                    # mid-stream: partition-broadcast on GpSimd (idle engine,
                    # zero PE/PSUM cost; latency hidden by the deferral)
                    bc_sb = att_pool.tile([64, 1024], f32, tag="bc",
                                          name="bc_sb", bufs=2)
                    nc.gpsimd.partition_broadcast(bc_sb[0:64, :],
                                                  recip[0:1, :])
                for h, c0 in ((fA, 0), (fB, 512)):
                    csl = slice(c0, c0 + 512)
                    if final:
                        # final tail: K=1 fp32 matmul on the now-idle PE is
                        # faster than waking GpSimd (~3us with its drain)
                        bc_ps = psum.tile([64, 512], f32, tag="ins",
                                          name="bc_ps",
                                          padded_shape=[128, 512], bufs=2)
                        nc.tensor.matmul(bc_ps[0:64, :], ones16[0:1, 0:64],
                                         recip[0:1, csl],
                                         start=True, stop=True)
                        bc_view = bc_ps[0:64, :]
                    else:
                        bc_view = bc_sb[0:64, csl]
                    nc.vector.tensor_mul(
                        o_sb[0:64, csl], av_sb[0:64, csl], bc_view)
                    # tail: scalar queue is idle once inputs are in; split
                    # the final out DMAs across both queues
                    q_eng = nc.scalar if (final and c0 == 0) else nc.sync
                    q_eng.dma_start(
                        outT[h * 64:h * 64 + 64, fqsl], o_sb[0:64, csl])

            # Flat 192-iteration stream: the S/exp pipeline runs a constant
            # 2 iterations ahead ACROSS qchunk and pair boundaries, so the
            # exp stream never rebuilds its lookahead at a boundary. The
            # AV/normalize stream is drained from a gated queue (AV_GATE) so
            # it tolerates DMA/eviction lag without stalling S/exp.
            # p tiles are a manual ring (not a pool): the ring recycle is a
            # plain WAR dep that is implied by each ACT's own S-matmul wait,
            # so no standalone pool-semaphore instruction burns ScalarE time.
            p_ring = [persist.tile([128, 1024], bf16, tag=f"pr{j}",
                                   name=f"pr{j}") for j in range(24)]
            p_tiles = {}

            def emit_s_exp(gi):
                if gi < 16:
                    with tc.high_priority():
                        _emit_s_exp(gi)
                    return
                _emit_s_exp(gi)

            def _emit_s_exp(gi):
                sp, r = divmod(gi, 64)
                sqch, skt = divmod(r, 16)
                qsl = slice(sqch * 512, (sqch + 1) * 512)
                ksl = slice(skt * 128, (skt + 1) * 128)
                s_AB = psum.tile([128, 1024], f32, tag="s",
                                 name="sAB", bufs=2)
                nc.tensor.matmul(
                    s_AB[:, 0:512],
                    wkT_sb[sp][0:64, ksl], wqT_sb[sp][0:64, qsl],
                    start=True, stop=True,
                    tile_position=(0, 0) if _TILEPOS else None,
                )
                nc.tensor.matmul(
                    s_AB[:, 512:1024],
                    wkT_sb[sp][64:128, ksl], wqT_sb[sp][64:128, qsl],
                    start=True, stop=True,
                    tile_position=(64, 0) if _TILEPOS else None,
                )
                p_AB = p_ring[gi % 24]
                nc.scalar.activation(p_AB[:, :], s_AB[:, :], AF.Exp)
                p_tiles[gi] = p_AB

            # per-qchunk AV accumulator state
            av_state = {"avA": None, "avB": None}
            next_av = [0]  # next global AV index to emit

            def emit_av(k):
                qi, kt = divmod(k, _ST)
                p, qch = divmod(qi, _QC)
                hA, hB = 2 * p, 2 * p + 1
                if kt == 0:
                    # two 1-bank accumulators: the eviction copy of half A
                    # can overlap AV15 of half B, and next qchunk's AV0.A
                    # only waits on copyA - shrinks the boundary stall
                    av_state["avA"] = psum.tile(
                        [65, 512], f32, tag="avA", name="avA",
                        padded_shape=[128, 512], bufs=1)
                    av_state["avB"] = psum.tile(
                        [65, 512], f32, tag="avB", name="avB",
                        padded_shape=[128, 512], bufs=1)
                avA, avB = av_state["avA"], av_state["avB"]
                pv = p_tiles.pop(k)
                nc.tensor.matmul(
                    avA[0:65, :], wv_sb[:, kt, hA, 0:65], pv[:, 0:512],
                    start=(kt == 0), stop=(kt == _ST - 1),
                )
                nc.tensor.matmul(
                    avB[0:65, :], wv_sb[:, kt, hB, 0:65], pv[:, 512:1024],
                    start=(kt == 0), stop=(kt == _ST - 1),
                )
                if kt != _ST - 1:
                    return
                # ---- qchunk done: evict + reciprocal (latency-tolerant) ----
                qsl = slice(qch * 512, (qch + 1) * 512)
                av_sb = att_pool.tile([65, 1024], f32, tag="av_sb",
                                      name="av_sb", bufs=2)
                last = k == N_GI - 1
                with tc.high_priority():
                    if last:
                        # final tail: sums rows only; the big value copies
                        # are emitted AFTER the reciprocal bounce below so
                        # the DVE runs the recips as soon as the bounce DMA
                        # lands instead of behind the copies
                        nc.vector.tensor_copy(av_sb[64:65, 512:1024],
                                              avB[64:65, :])
                        nc.vector.tensor_copy(av_sb[64:65, 0:512],
                                              avA[64:65, :])
                    else:
                        # mid-stream: A first - next qchunk's AV0.A is the
                        # head of the PE stream and waits only on copyA
                        nc.vector.tensor_copy(av_sb[0:65, 0:512],
                                              avA[0:65, :])
                        nc.vector.tensor_copy(av_sb[0:65, 512:1024],
                                              avB[0:65, :])
                # reciprocal is partition-serial on DVE; bounce the sums
                # through [128,4] so all lanes work, then bounce back.
                rp = att_pool.tile([128, 8], f32, tag="rp", name="rp",
                                   bufs=2)
                if last:
                    # bf16 recip: the final-flush broadcast matmul runs
                    # single-pass bf16 instead of two-pass fp32 (~1.5us off
                    # the serial tail; ~0.4% on the denominator is inside
                    # the accuracy budget)
                    rp2 = att_pool.tile([128, 8], bf16, tag="rp2f",
                                        name="rp2f", bufs=1)
                    recip = att_pool.tile([1, 1024], bf16, tag="recipf",
                                          name="recipf", bufs=1)
                else:
                    rp2 = att_pool.tile([128, 8], f32, tag="rp2", name="rp2",
                                        bufs=2)
                    recip = att_pool.tile([1, 1024], f32, tag="recip",
                                          name="recip", bufs=2)
                bq_eng = nc.scalar if last else nc.sync
                for c0 in (0, 512):
                    csl = slice(c0, c0 + 512)
                    rsl = slice(c0 // 128, c0 // 128 + 4)
                    bq_eng.dma_start(rp[0:128, rsl], av_sb[64:65, csl])
                    with nc.allow_low_precision(reason="bf16 recip bcast"):
                        nc.vector.reciprocal(rp2[0:128, rsl], rp[0:128, rsl])
                    bq_eng.dma_start(recip[0:1, csl], rp2[0:128, rsl])
                if last:
                    # deferred big value copies (only the muls need them)
                    nc.vector.tensor_copy(av_sb[0:64, 512:1024], avB[0:64, :])
                    nc.vector.tensor_copy(av_sb[0:64, 0:512], avA[0:64, :])
                pending.append((av_sb, recip, hA, hB, qsl))

            emit_s_exp(0)
            emit_s_exp(1)
            for gi in range(N_GI):
                if gi + 2 < N_GI:
                    emit_s_exp(gi + 2)
                if pending and flush_gate[0] is not None and gi >= flush_gate[0]:
                    flush_pending()
                    flush_gate[0] = None
                for item in INSERTS.get(gi, ()):
                    emit_insert(item)
                while next_av[0] < N_GI and next_av[0] <= gi \
                        and AV_GATE[next_av[0]] <= gi:
                    k = next_av[0]
                    emit_av(k)
                    if k % _ST == _ST - 1:
                        flush_gate[0] = gi + 3
                    next_av[0] += 1

            # drain any AVs still gated past the loop end
            while next_av[0] < N_GI:
                if pending and len(pending) > 1:
                    flush_pending()
                emit_av(next_av[0])
                next_av[0] += 1
            while len(pending) > 1:
                flush_pending()
            with tc.high_priority():
                flush_pending(final=True)

    nc.compile()
    return nc


def _get_compiled():
    global _compiled
    if _compiled is None:
        _compiled = _build()
    return _compiled


def make_in_maps(q, k, v, Wq, bq, Wk, bk, Wv, bv):
    import ml_dtypes

    bf16 = ml_dtypes.bfloat16

    def pack_x(xT):  # [768, 2048] -> [128, 4nch, 6kt, 512]
        return np.ascontiguousarray(
            xT.reshape(_KT, 128, 4, 512).transpose(1, 2, 0, 3)).astype(bf16)

    def pack_w(WT):  # [768, 384] -> [128, 3m, 6kt, 128]
        return np.ascontiguousarray(
            WT.reshape(_KT, 128, _NP, 128).transpose(1, 2, 0, 3)).astype(bf16)

    in_maps = []
    for c in range(_NCORES):
        b, g = c // 2, c % 2
        gsl = slice(g * _GSZ, (g + 1) * _GSZ)
        b_pack = np.concatenate([
            np.asarray(bq)[gsl].reshape(3, 128).T,
            np.asarray(bk)[gsl].reshape(3, 128).T,
            np.tile(np.asarray(bv)[gsl][None, :], (128, 1)),
        ], axis=1).astype(np.float32)
        in_maps.append({
            "q_pk": pack_x(np.asarray(q)[b].T),
            "k_pk": pack_x(np.asarray(k)[b].T),
            "v_pk": pack_x(np.asarray(v)[b].T),
            "Wq_pk": pack_w(np.asarray(Wq)[gsl, :].T),
            "Wk_pk": pack_w(np.asarray(Wk)[gsl, :].T),
            "Wv_pk": np.ascontiguousarray(np.asarray(Wv)[gsl, :].T.reshape(
                _KT, 128, _GSZ).transpose(1, 0, 2)).astype(bf16),
            "b_all": np.ascontiguousarray(b_pack),
        })
    return in_maps


def assemble_out(results):
    out = np.zeros((_BS, _SEQ, _DM), np.float32)
    for c in range(_NCORES):
        b, g = c // 2, c % 2
        out[b, :, g * _GSZ:(g + 1) * _GSZ] = np.asarray(
            results[c]["outT"], np.float32
        ).T
    return out


def kernel(q, k, v, Wq, bq, Wk, bk, Wv, bv):
    from concourse.bass_utils import run_bass_kernel_spmd

    nc = _get_compiled()
    in_maps = make_in_maps(q, k, v, Wq, bq, Wk, bk, Wv, bv)
    res = run_bass_kernel_spmd(nc, in_maps, core_ids=list(range(_NCORES)))
    return assemble_out(res.results)


# revision 9
# speedup vs baseline: 1.0082x; 1.0082x over previous
"""Multi-head attention (QKV proj + softmax(QK^T)V) on 8 TRN2 NeuronCores.

Sharding: 8 cores = 4 batches x 2 head-groups (6 heads each). Pure data
parallel - no collectives. Host pre-transposes shards so every on-device
matmul streams with zero on-chip transposes.

The kernel is PE-bound: S (65us) + AV (104us) + projections (52us) =
~220us of tensor-engine work vs 200us of ScalarE exp. Everything is
scheduled so the in-order PE never waits:
  - input DMAs kt-halved and interleaved across BOTH hwdge queues in
    deadline order, so each tensor's halves land concurrently and the
    head projections can start on the first half
  - PE warmup matmuls at t=0 beat the DVFS ramp (full clock needs ~3us
    of continuous execution) so the head projections run at speed
  - per-core pipeline (transposed layouts, d-on-partitions):
      wqT = WqT.T @ qT + bq   [384,2048] (pair p -> head 2p rows 0:64, 2p+1 rows 64:128)
      wv  = vT.T @ WvT + bv   (stored per seq-tile, ones column rides col 64)
      S^T = wkT.T @ wqT per head pair via PE quadrants (tile_position)
      exp on ScalarE (no max subtraction; scores <~70, fp32-safe)
      [out.T; rowsum] = [wv | 1].T @ P^T  (denominator rides the AV matmul)
  - projection units are inserted on a deadline schedule derived from a
    DMA bandwidth model; first-qchunk AV pairs are GATED (deferred up to
    the 24-slot p-ring depth) so late v chunks never stall the PE
  - normalize: recip on DVE via [128,8] bounce (partition-serial otherwise),
    partition-broadcast on GpSimd mid-stream / K=1 matmul at the tail,
    final flush runs at high priority on the then-idle scalar queue.
"""

import sys

if "/opt/trn_rl_repo" not in sys.path:
    sys.path.insert(0, "/opt/trn_rl_repo")

import numpy as np

_TILEPOS = True

_BS, _SEQ, _DM = 4, 2048, 768
_NH, _DH = 12, 64
_GSZ = _DM // 2  # 384 dims per head-group
_NCORES = 8

_KT = _DM // 128   # 6 contraction tiles
_ST = _SEQ // 128  # 16 seq tiles (key positions)
_QC = _SEQ // 512  # 4 query chunks
_NP = _GSZ // 128  # 3 head pairs

_compiled = None


def _insert_schedule():
    """global-iter -> list of insert items. iter = 64*p + 16*qch + kt.

    Items: ("V", st, 0) one V-projection unit (6 matmuls of 384 cols);
           ("qk", which, m, nch, c0, w) q/k unit (6 matmuls of w cols).
    Placement follows a DMA model (two hwdge queues, deadline-ordered
    kt-halved transfers, ~360-490 KB/us aggregate) so each unit's data
    is present when the in-order PE reaches it.
    """
    ins = {}

    def add(it, item):
        ins.setdefault(it, []).append(item)

    # pair-0 k/q: k(0,nch) must precede S kt=4nch (emitted at gi 4nch-2);
    # q(0,c) before S of qchunk c (emitted at gi 16c-2).
    add(1, ("qk", "k", 0, 1, 0, 512))
    add(5, ("qk", "k", 0, 2, 0, 512))
    add(9, ("qk", "k", 0, 3, 0, 512))
    add(13, ("qk", "q", 0, 1, 0, 512))
    add(29, ("qk", "q", 0, 2, 0, 512))
    add(45, ("qk", "q", 0, 3, 0, 512))
    # V units: gated AVs (see _AV_GATE) tolerate late placement; spread
    # per the v-chunk DMA arrivals (v lands in two 2-chunk transfers).
    for st in range(4):
        add(10 + st, ("V", st, 0))
    for st in range(4, 8):
        add(14 + (st - 4), ("V", st, 0))
    for st in range(8, 12):
        add(18 + (st - 8), ("V", st, 0))
    for st in range(12, 16):
        add(22 + (st - 12), ("V", st, 0))
    # pair 1 (deadlines: k(1,n) by 61+4n, q(1,c) by 61+16c)
    for nch in range(4):
        add(46 + 3 * nch, ("qk", "k", 1, nch, 0, 512))
    add(59, ("qk", "q", 1, 0, 0, 512))
    for qch, base in ((1, 74), (2, 90), (3, 105)):
        add(base, ("qk", "q", 1, qch, 0, 512))
    # pair 2 (deadlines: k(2,n) by 125+4n, q(2,c) by 125+16c)
    for nch in range(4):
        add(110 + 3 * nch, ("qk", "k", 2, nch, 0, 512))
    add(122, ("qk", "q", 2, 0, 0, 512))
    for qch, base in ((1, 138), (2, 154), (3, 170)):
        add(base, ("qk", "q", 2, qch, 0, 512))

    # safety: every unit placed before its consumer
    for it, items in ins.items():
        for item in items:
            if item[0] == "V":
                pass  # V deadline enforced via _AV_GATE
            else:
                _, which, m, nch, c0, w = item
                if which == "k":
                    dl = 64 * m + 4 * nch - 2
                else:
                    dl = 64 * m + 16 * nch - 2
                assert it <= max(dl, 1) or m == 0 and nch <= 1, (it, item)
    return ins


def _av_gates(inserts):
    """Earliest gi at which AV pair k (global 0..191) may be emitted.

    qchunk 0 (pair 0): gated one iter after its V insert so a late v DMA
    stalls only the latency-tolerant AV stream, not S/exp. Later
    qchunks: classic 2-deferral (boundary S-pairs run ahead of the
    accumulator-eviction wait).
    """
    v_gi = {}
    for gi, items in inserts.items():
        for item in items:
            if item[0] == "V":
                v_gi[item[1]] = gi
    gates = []
    for k in range(_NP * _QC * _ST):
        qi, kt = divmod(k, _ST)
        if qi == 0:
            gates.append(v_gi[kt] + 1)
        else:
            gates.append(16 * qi + max(kt, 2))
    return gates


def _build():
    import concourse.bass as bass  # noqa: F401
    import concourse.mybir as mybir
    import concourse.tile as tile
    from concourse import bacc

    f32 = mybir.dt.float32
    bf16 = mybir.dt.bfloat16
    AF = mybir.ActivationFunctionType

    nc = bacc.Bacc("TRN2", target_bir_lowering=False, debug=False)

    # q/k/v packed [p, nch, kt, c]: one nch slice = 128 descriptors x 6KB
    q_pk = nc.dram_tensor("q_pk", [128, 4, _KT, 512], bf16, kind="ExternalInput")
    k_pk = nc.dram_tensor("k_pk", [128, 4, _KT, 512], bf16, kind="ExternalInput")
    v_pk = nc.dram_tensor("v_pk", [128, 4, _KT, 512], bf16, kind="ExternalInput")
    # Wq/Wk packed [p, m, kt, c]: one m slab = 128 x 1.5KB (m=0 slabs ride
    # W0b_pk instead; only m=1,2 are transferred from these)
    Wq_pk = nc.dram_tensor("Wq_pk", [128, _NP, _KT, 128], bf16, kind="ExternalInput")
    Wk_pk = nc.dram_tensor("Wk_pk", [128, _NP, _KT, 128], bf16, kind="ExternalInput")
    Wv_pk = nc.dram_tensor("Wv_pk", [128, _KT, _GSZ], bf16, kind="ExternalInput")
    # the hwdge queue is descriptor-count-bound (~0.65us + ~25ns/descr: any
    # 128-partition transfer costs ~3.2us regardless of bytes), so the
    # ramp-critical Wk0+Wq0 slabs AND all biases (bf16) ride ONE transfer:
    # cols [0:768]=Wk0, [768:1536]=Wq0, [1536:1542]=bqkT, [1542:1926]=bv
    W0b_pk = nc.dram_tensor("W0b_pk", [128, 1926], bf16, kind="ExternalInput")
    outT = nc.dram_tensor("outT", [_GSZ, _SEQ], f32, kind="ExternalOutput")

    INSERTS = _insert_schedule()
    AV_GATE = _av_gates(INSERTS)
    N_GI = _NP * _QC * _ST

    with tile.TileContext(nc) as tc:
        with (
            tc.tile_pool(name="persist", bufs=1) as persist,
            tc.tile_pool(name="qkv", bufs=1) as qkv_pool,
            tc.tile_pool(name="w", bufs=1) as w_pool,
            tc.tile_pool(name="psum", bufs=2, space="PSUM") as psum,
            tc.tile_pool(name="att", bufs=4) as att_pool,
        ):
            # ---- persistent SBUF ----
            wqT_sb = [persist.tile([128, _SEQ], bf16, tag=f"wqT{p}", name=f"wqT{p}")
                      for p in range(_NP)]
            wkT_sb = [persist.tile([128, _SEQ], bf16, tag=f"wkT{p}", name=f"wkT{p}")
                      for p in range(_NP)]
            warm_sb = persist.tile([1, 512], bf16, tag="warm")
            ones16 = persist.tile([1, 64], bf16, tag="ones16")
            with tc.high_priority():
                nc.vector.memset(warm_sb[:, :], 0.0)
                nc.vector.memset(ones16[:, :], 1.0)
            # per seq-tile, per head: [64 wv dims | ones | pad]
            wv_sb = persist.tile([128, _ST, 6, 66], bf16, tag="wv")
            for st in range(_ST):
                nc.vector.memset(wv_sb[:, st, :, 64:65], 1.0)

            q_all = qkv_pool.tile([128, 4, _KT, 512], bf16, tag="qa", name="q_all")
            k_all = qkv_pool.tile([128, 4, _KT, 512], bf16, tag="ka", name="k_all")
            v_all = qkv_pool.tile([128, 4, _KT, 512], bf16, tag="va", name="v_all")
            wq_all = w_pool.tile([128, _NP - 1, _KT, 128], bf16, tag="wqa", name="wq_all")
            wk_all = w_pool.tile([128, _NP - 1, _KT, 128], bf16, tag="wka", name="wk_all")
            wv_all = w_pool.tile([128, _KT, _GSZ], bf16, tag="wva", name="wv_all")
            w0_sb = persist.tile([128, 1926], bf16, tag="w0")
            b_sb = persist.tile([128, 6 + _GSZ], f32, tag="b")

            # ---- input DMAs: descriptor-count-minimized, interleaved
            # across both queues in deadline order. Each 128-partition
            # transfer costs ~3.2us of queue time no matter its size, so v
            # chunks ride in contiguous pairs and W0+biases in one shot.
            nc.scalar.dma_start(w0_sb[:, :], W0b_pk[:, :])
            nc.sync.dma_start(k_all[:, 0], k_pk[:, 0])
            nc.scalar.dma_start(q_all[:, 0], q_pk[:, 0])
            nc.sync.dma_start(k_all[:, 1], k_pk[:, 1])
            nc.scalar.dma_start(wv_all[:, :], Wv_pk[:, :])
            nc.sync.dma_start(k_all[:, 2], k_pk[:, 2])
            nc.scalar.dma_start(q_all[:, 1], q_pk[:, 1])
            nc.sync.dma_start(k_all[:, 3], k_pk[:, 3])
            nc.scalar.dma_start(v_all[:, 0:2], v_pk[:, 0:2])
            nc.sync.dma_start(q_all[:, 2], q_pk[:, 2])
            nc.scalar.dma_start(v_all[:, 2:4], v_pk[:, 2:4])
            nc.sync.dma_start(q_all[:, 3], q_pk[:, 3])
            nc.sync.dma_start(wk_all[:, 0], Wk_pk[:, 1])
            nc.scalar.dma_start(wq_all[:, 0], Wq_pk[:, 1])
            nc.sync.dma_start(wk_all[:, 1], Wk_pk[:, 2])
            nc.scalar.dma_start(wq_all[:, 1], Wq_pk[:, 2])

            # biases: bf16 in W0b -> f32 working copy (tiny DVE copies)
            with tc.high_priority():
                nc.vector.tensor_copy(b_sb[:, 0:6], w0_sb[:, 1536:1542])
                nc.vector.tensor_copy(b_sb[:, 6:6 + _GSZ],
                                      w0_sb[:, 1542:1926])

            # ---- PE warmup: the tensor engine needs ~3us of continuous
            # execution to reach full clock. Dummy matmuls (zeros moving)
            # fill the k0-DMA wait so the head projections run at speed.
            with tc.high_priority():
                for wi in range(4):
                    wps = psum.tile([64, 512], f32, tag="ins", name="warm_ps",
                                    padded_shape=[128, 512], bufs=2)
                    nc.tensor.matmul(wps[0:64, :], ones16[0:1, 0:64],
                                     warm_sb[0:1, :], start=True, stop=True)

            # ---- projection unit emitters ----
            def emit_v_unit(st, pr, npair=1):
                nch, cc = st // 4, (st % 4) * 128
                d0, dn = 128 * pr, 128 * npair
                nh = 2 * npair
                psv = psum.tile([128, dn], f32, tag="ins", name="psv",
                                padded_shape=[128, 512], bufs=2)
                for t in range(_KT):
                    nc.tensor.matmul(
                        psv[:, :], v_all[:, nch, t, cc:cc + 128],
                        wv_all[:, t, d0:d0 + dn],
                        start=(t == 0), stop=(t == _KT - 1),
                    )
                nc.vector.tensor_add(
                    wv_sb[:, st, 2 * pr:2 * pr + nh, 0:64],
                    psv[:, :].rearrange("p (h d) -> p h d", h=nh),
                    b_sb[:, 6 + d0:6 + d0 + dn].rearrange(
                        "p (h d) -> p h d", h=nh),
                )

            def emit_qk_part(which, m, nch, c0, w):
                ps = psum.tile([128, w], f32, tag="ins", name="psqk",
                               padded_shape=[128, 512], bufs=2)
                x_all, w_all, dst, boff, w0off = (
                    (q_all, wq_all, wqT_sb, 0, 768) if which == "q"
                    else (k_all, wk_all, wkT_sb, 3, 0)
                )
                for t in range(_KT):
                    wap = (w0_sb[:, w0off + t * 128:w0off + (t + 1) * 128]
                           if m == 0 else w_all[:, m - 1, t, :])
                    nc.tensor.matmul(
                        ps[:, :], wap,
                        x_all[:, nch, t, c0:c0 + w],
                        start=(t == 0), stop=(t == _KT - 1),
                    )
                nc.vector.tensor_scalar_add(
                    dst[m][:, nch * 512 + c0:nch * 512 + c0 + w],
                    ps[:, :], b_sb[:, boff + m:boff + m + 1])

            def emit_insert(item):
                if item[0] == "V":
                    emit_v_unit(item[1], item[2], npair=3)
                else:
                    _, which, m, nch, c0, w = item
                    emit_qk_part(which, m, nch, c0, w)

            # ---- head: pair-0 qchunk-0 q/k projections ----
            # SPINE priority offset: the attention spine (S-pairs, exps, and
            # the head k00/q00 that gate exp#1) is emitted with a large
            # priority offset so the Tile list-scheduler never displaces
            # ready spine work with projection fillers whose DMA data may be
            # late on real hardware.
            with tc.high_priority():
                emit_qk_part("k", 0, 0, 0, 512)
                emit_qk_part("q", 0, 0, 0, 512)

            # ---- attention stream ----
            # deferred normalize state: (av_sb, recip, hA, hB, qsl) of the
            # previous qchunk; its broadcast + muls + out DMAs are emitted a
            # few iterations later so the in-order PE stream never stalls on
            # the reciprocal round-trip at a boundary.
            pending = []
            flush_gate = [None]  # gi at which pending may flush

            def flush_pending(final=False):
                av_sb, recip, fA, fB, fqsl = pending.pop()
                o_sb = att_pool.tile([64, 1024], f32, tag="o", name="o_sb",
                                     bufs=2)
                if not final:
                    # mid-stream: partition-broadcast on GpSimd (idle engine,
                    # zero PE/PSUM cost; latency hidden by the deferral)
                    bc_sb = att_pool.tile([64, 1024], f32, tag="bc",
                                          name="bc_sb", bufs=2)
                    nc.gpsimd.partition_broadcast(bc_sb[0:64, :],
                                                  recip[0:1, :])
                for h, c0 in ((fA, 0), (fB, 512)):
                    csl = slice(c0, c0 + 512)
                    if final:
                        # final tail: K=1 fp32 matmul on the now-idle PE is
                        # faster than waking GpSimd (~3us with its drain)
                        bc_ps = psum.tile([64, 512], f32, tag="ins",
                                          name="bc_ps",
                                          padded_shape=[128, 512], bufs=2)
                        nc.tensor.matmul(bc_ps[0:64, :], ones16[0:1, 0:64],
                                         recip[0:1, csl],
                                         start=True, stop=True)
                        bc_view = bc_ps[0:64, :]
                    else:
                        bc_view = bc_sb[0:64, csl]
                    nc.vector.tensor_mul(
                        o_sb[0:64, csl], av_sb[0:64, csl], bc_view)
                    # tail: scalar queue is idle once inputs are in; split
                    # the final out DMAs across both queues
                    q_eng = nc.scalar if (final and c0 == 0) else nc.sync
                    q_eng.dma_start(
                        outT[h * 64:h * 64 + 64, fqsl], o_sb[0:64, csl])

            # Flat 192-iteration stream: the S/exp pipeline runs a constant
            # 2 iterations ahead ACROSS qchunk and pair boundaries, so the
            # exp stream never rebuilds its lookahead at a boundary. The
            # AV/normalize stream is drained from a gated queue (AV_GATE) so
            # it tolerates DMA/eviction lag without stalling S/exp.
            # p tiles are a manual ring (not a pool): the ring recycle is a
            # plain WAR dep that is implied by each ACT's own S-matmul wait,
            # so no standalone pool-semaphore instruction burns ScalarE time.
            p_ring = [persist.tile([128, 1024], bf16, tag=f"pr{j}",
                                   name=f"pr{j}") for j in range(24)]
            p_tiles = {}

            def emit_s_exp(gi):
                if gi < 16:
                    with tc.high_priority():
                        _emit_s_exp(gi)
                    return
                _emit_s_exp(gi)

            def _emit_s_exp(gi):
                sp, r = divmod(gi, 64)
                sqch, skt = divmod(r, 16)
                qsl = slice(sqch * 512, (sqch + 1) * 512)
                ksl = slice(skt * 128, (skt + 1) * 128)
                s_AB = psum.tile([128, 1024], f32, tag="s",
                                 name="sAB", bufs=2)
                nc.tensor.matmul(
                    s_AB[:, 0:512],
                    wkT_sb[sp][0:64, ksl], wqT_sb[sp][0:64, qsl],
                    start=True, stop=True,
                    tile_position=(0, 0) if _TILEPOS else None,
                )
                nc.tensor.matmul(
                    s_AB[:, 512:1024],
                    wkT_sb[sp][64:128, ksl], wqT_sb[sp][64:128, qsl],
                    start=True, stop=True,
                    tile_position=(64, 0) if _TILEPOS else None,
                )
                p_AB = p_ring[gi % 24]
                nc.scalar.activation(p_AB[:, :], s_AB[:, :], AF.Exp)
                p_tiles[gi] = p_AB

            # per-qchunk AV accumulator state
            av_state = {"avA": None, "avB": None}
            next_av = [0]  # next global AV index to emit

            def emit_av(k):
                qi, kt = divmod(k, _ST)
                p, qch = divmod(qi, _QC)
                hA, hB = 2 * p, 2 * p + 1
                if kt == 0:
                    # two 1-bank accumulators: the eviction copy of half A
                    # can overlap AV15 of half B, and next qchunk's AV0.A
                    # only waits on copyA - shrinks the boundary stall
                    av_state["avA"] = psum.tile(
                        [65, 512], f32, tag="avA", name="avA",
                        padded_shape=[128, 512], bufs=1)
                    av_state["avB"] = psum.tile(
                        [65, 512], f32, tag="avB", name="avB",
                        padded_shape=[128, 512], bufs=1)
                avA, avB = av_state["avA"], av_state["avB"]
                pv = p_tiles.pop(k)
                nc.tensor.matmul(
                    avA[0:65, :], wv_sb[:, kt, hA, 0:65], pv[:, 0:512],
                    start=(kt == 0), stop=(kt == _ST - 1),
                )
                nc.tensor.matmul(
                    avB[0:65, :], wv_sb[:, kt, hB, 0:65], pv[:, 512:1024],
                    start=(kt == 0), stop=(kt == _ST - 1),
                )
                if kt != _ST - 1:
                    return
                # ---- qchunk done: evict + reciprocal (latency-tolerant) ----
                qsl = slice(qch * 512, (qch + 1) * 512)
                av_sb = att_pool.tile([65, 1024], f32, tag="av_sb",
                                      name="av_sb", bufs=2)
                last = k == N_GI - 1
                with tc.high_priority():
                    if last:
                        # final tail: sums rows only; the big value copies
                        # are emitted AFTER the reciprocal bounce below so
                        # the DVE runs the recips as soon as the bounce DMA
                        # lands instead of behind the copies
                        nc.vector.tensor_copy(av_sb[64:65, 512:1024],
                                              avB[64:65, :])
                        nc.vector.tensor_copy(av_sb[64:65, 0:512],
                                              avA[64:65, :])
                    else:
                        # mid-stream: A first - next qchunk's AV0.A is the
                        # head of the PE stream and waits only on copyA
                        nc.vector.tensor_copy(av_sb[0:65, 0:512],
                                              avA[0:65, :])
                        nc.vector.tensor_copy(av_sb[0:65, 512:1024],
                                              avB[0:65, :])
                # reciprocal is partition-serial on DVE; bounce the sums
                # through [128,4] so all lanes work, then bounce back.
                rp = att_pool.tile([128, 8], f32, tag="rp", name="rp",
                                   bufs=2)
                if last:
                    # bf16 recip: the final-flush broadcast matmul runs
                    # single-pass bf16 instead of two-pass fp32 (~1.5us off
                    # the serial tail; ~0.4% on the denominator is inside
                    # the accuracy budget)
                    rp2 = att_pool.tile([128, 8], bf16, tag="rp2f",
                                        name="rp2f", bufs=1)
                    recip = att_pool.tile([1, 1024], bf16, tag="recipf",
                                          name="recipf", bufs=1)
                else:
                    rp2 = att_pool.tile([128, 8], f32, tag="rp2", name="rp2",
                                        bufs=2)
                    recip = att_pool.tile([1, 1024], f32, tag="recip",
                                          name="recip", bufs=2)
                bq_eng = nc.scalar if last else nc.sync
                for c0 in (0, 512):
                    csl = slice(c0, c0 + 512)
                    rsl = slice(c0 // 128, c0 // 128 + 4)
                    bq_eng.dma_start(rp[0:128, rsl], av_sb[64:65, csl])
                    with nc.allow_low_precision(reason="bf16 recip bcast"):
                        nc.vector.reciprocal(rp2[0:128, rsl], rp[0:128, rsl])
                    bq_eng.dma_start(recip[0:1, csl], rp2[0:128, rsl])
                if last:
                    # deferred big value copies (only the muls need them)
                    nc.vector.tensor_copy(av_sb[0:64, 512:1024], avB[0:64, :])
                    nc.vector.tensor_copy(av_sb[0:64, 0:512], avA[0:64, :])
                pending.append((av_sb, recip, hA, hB, qsl))

            emit_s_exp(0)
            emit_s_exp(1)
            for gi in range(N_GI):
                if gi + 2 < N_GI:
                    emit_s_exp(gi + 2)
                if pending and flush_gate[0] is not None and gi >= flush_gate[0]:
                    flush_pending()
                    flush_gate[0] = None
                for item in INSERTS.get(gi, ()):
                    emit_insert(item)
                ndr = 0
                while next_av[0] < N_GI and next_av[0] <= gi \
                        and AV_GATE[next_av[0]] <= gi and ndr < 3:
                    k = next_av[0]
                    emit_av(k)
                    if k % _ST == _ST - 1:
                        flush_gate[0] = gi + 3
                    next_av[0] += 1
                    ndr += 1

            # drain any AVs still gated past the loop end
            while next_av[0] < N_GI:
                if pending and len(pending) > 1:
                    flush_pending()
                emit_av(next_av[0])
                next_av[0] += 1
            while len(pending) > 1:
                flush_pending()
            with tc.high_priority():
                flush_pending(final=True)

    nc.compile()
    return nc


def _get_compiled():
    global _compiled
    if _compiled is None:
        _compiled = _build()
    return _compiled


def make_in_maps(q, k, v, Wq, bq, Wk, bk, Wv, bv):
    import ml_dtypes

    bf16 = ml_dtypes.bfloat16

    def pack_x(xT):  # [768, 2048] -> [128, 4nch, 6kt, 512]
        return np.ascontiguousarray(
            xT.reshape(_KT, 128, 4, 512).transpose(1, 2, 0, 3)).astype(bf16)

    def pack_w(WT):  # [768, 384] -> [128, 3m, 6kt, 128]
        return np.ascontiguousarray(
            WT.reshape(_KT, 128, _NP, 128).transpose(1, 2, 0, 3)).astype(bf16)

    in_maps = []
    for c in range(_NCORES):
        b, g = c // 2, c % 2
        gsl = slice(g * _GSZ, (g + 1) * _GSZ)
        wq_p = pack_w(np.asarray(Wq)[gsl, :].T)
        wk_p = pack_w(np.asarray(Wk)[gsl, :].T)
        # one ramp-critical transfer: [Wk0 | Wq0 | bqT | bkT | bv] bf16
        w0b = np.concatenate([
            wk_p[:, 0].reshape(128, _KT * 128),
            wq_p[:, 0].reshape(128, _KT * 128),
            np.asarray(bq)[gsl].reshape(3, 128).T.astype(bf16),
            np.asarray(bk)[gsl].reshape(3, 128).T.astype(bf16),
            np.tile(np.asarray(bv)[gsl][None, :].astype(bf16), (128, 1)),
        ], axis=1)
        in_maps.append({
            "q_pk": pack_x(np.asarray(q)[b].T),
            "k_pk": pack_x(np.asarray(k)[b].T),
            "v_pk": pack_x(np.asarray(v)[b].T),
            "Wq_pk": wq_p,
            "Wk_pk": wk_p,
            "Wv_pk": np.ascontiguousarray(np.asarray(Wv)[gsl, :].T.reshape(
                _KT, 128, _GSZ).transpose(1, 0, 2)).astype(bf16),
            "W0b_pk": np.ascontiguousarray(w0b),
        })
    return in_maps


def assemble_out(results):
    out = np.zeros((_BS, _SEQ, _DM), np.float32)
    for c in range(_NCORES):
        b, g = c // 2, c % 2
        out[b, :, g * _GSZ:(g + 1) * _GSZ] = np.asarray(
            results[c]["outT"], np.float32
        ).T
    return out


def kernel(q, k, v, Wq, bq, Wk, bk, Wv, bv):
    from concourse.bass_utils import run_bass_kernel_spmd

    nc = _get_compiled()
    in_maps = make_in_maps(q, k, v, Wq, bq, Wk, bk, Wv, bv)
    res = run_bass_kernel_spmd(nc, in_maps, core_ids=list(range(_NCORES)))
    return assemble_out(res.results)


# revision 11
# speedup vs baseline: 1.0688x; 1.0602x over previous
"""Multi-head attention (QKV proj + softmax(QK^T)V) on 8 TRN2 NeuronCores.

Sharding: 8 cores = 4 batches x 2 head-groups (6 heads each). Pure data
parallel - no collectives. Host pre-transposes shards so every on-device
matmul streams with zero on-chip transposes.

The kernel is PE-bound: S (65us) + AV (104us) + projections (52us) =
~220us of tensor-engine work vs 200us of ScalarE exp. Everything is
scheduled so the in-order PE never waits:
  - input DMAs kt-halved and interleaved across BOTH hwdge queues in
    deadline order, so each tensor's halves land concurrently and the
    head projections can start on the first half
  - PE warmup matmuls at t=0 beat the DVFS ramp (full clock needs ~3us
    of continuous execution) so the head projections run at speed
  - per-core pipeline (transposed layouts, d-on-partitions):
      wqT = WqT.T @ qT + bq   [384,2048] (pair p -> head 2p rows 0:64, 2p+1 rows 64:128)
      wv  = vT.T @ WvT + bv   (stored per seq-tile, ones column rides col 64)
      S^T = wkT.T @ wqT per head pair via PE quadrants (tile_position)
      exp on ScalarE (no max subtraction; scores <~70, fp32-safe)
      [out.T; rowsum] = [wv | 1].T @ P^T  (denominator rides the AV matmul)
  - projection units are inserted on a deadline schedule derived from a
    DMA bandwidth model; first-qchunk AV pairs are GATED (deferred up to
    the 24-slot p-ring depth) so late v chunks never stall the PE
  - normalize: recip on DVE via [128,8] bounce (partition-serial otherwise),
    partition-broadcast on GpSimd mid-stream / K=1 matmul at the tail,
    final flush runs at high priority on the then-idle scalar queue.
"""

import sys

if "/opt/trn_rl_repo" not in sys.path:
    sys.path.insert(0, "/opt/trn_rl_repo")

import numpy as np

_TILEPOS = True

_BS, _SEQ, _DM = 4, 2048, 768
_NH, _DH = 12, 64
_GSZ = _DM // 2  # 384 dims per head-group
_NCORES = 8

_KT = _DM // 128   # 6 contraction tiles
_ST = _SEQ // 128  # 16 seq tiles (key positions)
_QC = _SEQ // 512  # 4 query chunks
_NP = _GSZ // 128  # 3 head pairs

_compiled = None


def _insert_schedule():
    """global-iter -> list of insert items. iter = 64*p + 16*qch + kt.

    Items: ("V", st, 0) one V-projection unit (6 matmuls of 384 cols);
           ("qk", which, m, nch, c0, w) q/k unit (6 matmuls of w cols).
    Placement follows a DMA model (two hwdge queues, deadline-ordered
    kt-halved transfers, ~360-490 KB/us aggregate) so each unit's data
    is present when the in-order PE reaches it.
    """
    ins = {}

    def add(it, item):
        ins.setdefault(it, []).append(item)

    # pair-0 k/q: k(0,nch) must precede S kt=4nch (emitted at gi 4nch-2);
    # q(0,c) before S of qchunk c (emitted at gi 16c-2).
    add(1, ("qk", "k", 0, 1, 0, 512))
    add(5, ("qk", "k", 0, 2, 0, 512))
    add(9, ("qk", "k", 0, 3, 0, 512))
    add(13, ("qk", "q", 0, 1, 0, 512))
    add(29, ("qk", "q", 0, 2, 0, 512))
    add(45, ("qk", "q", 0, 3, 0, 512))
    # V units: gated AVs (see _AV_GATE) tolerate late placement; spread
    # per the v-chunk DMA arrivals (v lands in two 2-chunk transfers).
    for st in range(16):
        add(5 + st, ("V", st, 0))
    # pair 1 (deadlines: k(1,n) by 61+4n, q(1,c) by 61+16c)
    for nch in range(4):
        add(46 + 3 * nch, ("qk", "k", 1, nch, 0, 512))
    add(59, ("qk", "q", 1, 0, 0, 512))
    for qch, base in ((1, 74), (2, 90), (3, 105)):
        add(base, ("qk", "q", 1, qch, 0, 512))
    # pair 2 (deadlines: k(2,n) by 125+4n, q(2,c) by 125+16c)
    for nch in range(4):
        add(110 + 3 * nch, ("qk", "k", 2, nch, 0, 512))
    add(122, ("qk", "q", 2, 0, 0, 512))
    for qch, base in ((1, 138), (2, 154), (3, 170)):
        add(base, ("qk", "q", 2, qch, 0, 512))

    # safety: every unit placed before its consumer
    for it, items in ins.items():
        for item in items:
            if item[0] == "V":
                pass  # V deadline enforced via _AV_GATE
            else:
                _, which, m, nch, c0, w = item
                if which == "k":
                    dl = 64 * m + 4 * nch - 2
                else:
                    dl = 64 * m + 16 * nch - 2
                assert it <= max(dl, 1) or m == 0 and nch <= 1, (it, item)
    return ins


def _av_gates(inserts):
    """Earliest gi at which AV pair k (global 0..191) may be emitted.

    qchunk 0 (pair 0): gated one iter after its V insert so a late v DMA
    stalls only the latency-tolerant AV stream, not S/exp. Later
    qchunks: classic 2-deferral (boundary S-pairs run ahead of the
    accumulator-eviction wait).
    """
    v_gi = {}
    for gi, items in inserts.items():
        for item in items:
            if item[0] == "V":
                v_gi[item[1]] = gi
    gates = []
    for k in range(_NP * _QC * _ST):
        qi, kt = divmod(k, _ST)
        if qi == 0:
            gates.append(v_gi[kt] + 1)
        else:
            gates.append(16 * qi + max(kt, 2))
    return gates


def _build():
    import concourse.bass as bass  # noqa: F401
    import concourse.mybir as mybir
    import concourse.tile as tile
    from concourse import bacc

    f32 = mybir.dt.float32
    bf16 = mybir.dt.bfloat16
    AF = mybir.ActivationFunctionType

    nc = bacc.Bacc("TRN2", target_bir_lowering=False, debug=False)

    # q/k/v packed [p, nch, kt, c]: one nch slice = 128 descriptors x 6KB
    q_pk = nc.dram_tensor("q_pk", [128, 4, _KT, 512], bf16, kind="ExternalInput")
    k_pk = nc.dram_tensor("k_pk", [128, 4, _KT, 512], bf16, kind="ExternalInput")
    v_pk = nc.dram_tensor("v_pk", [128, 4, _KT, 512], bf16, kind="ExternalInput")
    # Wq/Wk packed [p, m, kt, c]: one m slab = 128 x 1.5KB (m=0 slabs ride
    # W0b_pk instead; only m=1,2 are transferred from these)
    Wq_pk = nc.dram_tensor("Wq_pk", [128, _NP, _KT, 128], bf16, kind="ExternalInput")
    Wk_pk = nc.dram_tensor("Wk_pk", [128, _NP, _KT, 128], bf16, kind="ExternalInput")
    Wv_pk = nc.dram_tensor("Wv_pk", [128, _KT, _GSZ], bf16, kind="ExternalInput")
    # the hwdge queue is descriptor-count-bound (~0.65us + ~25ns/descr +
    # ~5us first-transfer latency: any 128-partition transfer costs ~3.2us
    # of queue time regardless of bytes), so EVERYTHING exp#1 needs rides
    # ONE transfer: cols [0:3072]=k chunk 0, [3072:3840]=Wk0,
    # [3840:4608]=Wq0, [4608:4614]=bqkT, [4614:4998]=bv
    W0b_pk = nc.dram_tensor("W0b_pk", [128, 4998], bf16, kind="ExternalInput")
    outT = nc.dram_tensor("outT", [_GSZ, _SEQ], f32, kind="ExternalOutput")

    INSERTS = _insert_schedule()
    AV_GATE = _av_gates(INSERTS)
    N_GI = _NP * _QC * _ST

    with tile.TileContext(nc) as tc:
        with (
            tc.tile_pool(name="persist", bufs=1) as persist,
            tc.tile_pool(name="qkv", bufs=1) as qkv_pool,
            tc.tile_pool(name="w", bufs=1) as w_pool,
            tc.tile_pool(name="psum", bufs=2, space="PSUM") as psum,
            tc.tile_pool(name="att", bufs=4) as att_pool,
        ):
            # ---- persistent SBUF ----
            wqT_sb = [persist.tile([128, _SEQ], bf16, tag=f"wqT{p}", name=f"wqT{p}")
                      for p in range(_NP)]
            wkT_sb = [persist.tile([128, _SEQ], bf16, tag=f"wkT{p}", name=f"wkT{p}")
                      for p in range(_NP)]
            warm_sb = persist.tile([1, 512], bf16, tag="warm")
            ones16 = persist.tile([1, 64], bf16, tag="ones16")
            with tc.high_priority():
                nc.vector.memset(warm_sb[:, :], 0.0)
                nc.vector.memset(ones16[:, :], 1.0)
            # per seq-tile, per head: [64 wv dims | ones | pad]
            wv_sb = persist.tile([128, _ST, 6, 66], bf16, tag="wv")
            for st in range(_ST):
                nc.vector.memset(wv_sb[:, st, :, 64:65], 1.0)

            q_all = qkv_pool.tile([128, 4, _KT, 512], bf16, tag="qa", name="q_all")
            k_all = qkv_pool.tile([128, 3, _KT, 512], bf16, tag="ka", name="k_all")
            v_all = qkv_pool.tile([128, 4, _KT, 512], bf16, tag="va", name="v_all")
            wq_all = w_pool.tile([128, _NP - 1, _KT, 128], bf16, tag="wqa", name="wq_all")
            wk_all = w_pool.tile([128, _NP - 1, _KT, 128], bf16, tag="wka", name="wk_all")
            wv_all = w_pool.tile([128, _KT, _GSZ], bf16, tag="wva", name="wv_all")
            w0_sb = persist.tile([128, 4998], bf16, tag="w0")
            b_sb = persist.tile([128, 6 + _GSZ], f32, tag="b")

            # ---- input DMAs: descriptor-count-minimized, interleaved
            # across both queues in deadline order. Each 128-partition
            # transfer costs ~3.2us of queue time no matter its size, so v
            # chunks ride in contiguous pairs and W0+biases in one shot.
            nc.scalar.dma_start(w0_sb[:, :], W0b_pk[:, :])
            nc.sync.dma_start(q_all[:, 0], q_pk[:, 0])
            nc.scalar.dma_start(wv_all[:, :], Wv_pk[:, :])
            nc.sync.dma_start(k_all[:, 0], k_pk[:, 1])
            nc.scalar.dma_start(v_all[:, 0:2], v_pk[:, 0:2])
            nc.sync.dma_start(k_all[:, 1], k_pk[:, 2])
            nc.scalar.dma_start(v_all[:, 2:4], v_pk[:, 2:4])
            nc.sync.dma_start(k_all[:, 2], k_pk[:, 3])
            nc.sync.dma_start(q_all[:, 1], q_pk[:, 1])
            nc.scalar.dma_start(wq_all[:, 0], Wq_pk[:, 1])
            nc.sync.dma_start(q_all[:, 2], q_pk[:, 2])
            nc.scalar.dma_start(wq_all[:, 1], Wq_pk[:, 2])
            nc.sync.dma_start(q_all[:, 3], q_pk[:, 3])
            nc.sync.dma_start(wk_all[:, 0], Wk_pk[:, 1])
            nc.sync.dma_start(wk_all[:, 1], Wk_pk[:, 2])

            # biases: bf16 in W0b -> f32 working copy (tiny DVE copies)
            with tc.high_priority():
                nc.vector.tensor_copy(b_sb[:, 0:6], w0_sb[:, 4608:4614])
                nc.vector.tensor_copy(b_sb[:, 6:6 + _GSZ],
                                      w0_sb[:, 4614:4998])

            # ---- PE warmup: the tensor engine needs ~3us of continuous
            # execution to reach full clock. Dummy matmuls (zeros moving)
            # fill the k0-DMA wait so the head projections run at speed.
            with tc.high_priority():
                for wi in range(14):
                    wps = psum.tile([64, 512], f32, tag="ins", name="warm_ps",
                                    padded_shape=[128, 512], bufs=2)
                    nc.tensor.matmul(wps[0:64, :], ones16[0:1, 0:64],
                                     warm_sb[0:1, :], start=True, stop=True)

            # ---- projection unit emitters ----
            def emit_v_unit(st, pr, npair=1):
                nch, cc = st // 4, (st % 4) * 128
                d0, dn = 128 * pr, 128 * npair
                nh = 2 * npair
                psv = psum.tile([128, dn], f32, tag="ins", name="psv",
                                padded_shape=[128, 512], bufs=2)
                for t in range(_KT):
                    nc.tensor.matmul(
                        psv[:, :], v_all[:, nch, t, cc:cc + 128],
                        wv_all[:, t, d0:d0 + dn],
                        start=(t == 0), stop=(t == _KT - 1),
                    )
                nc.vector.tensor_add(
                    wv_sb[:, st, 2 * pr:2 * pr + nh, 0:64],
                    psv[:, :].rearrange("p (h d) -> p h d", h=nh),
                    b_sb[:, 6 + d0:6 + d0 + dn].rearrange(
                        "p (h d) -> p h d", h=nh),
                )

            def emit_qk_part(which, m, nch, c0, w):
                ps = psum.tile([128, w], f32, tag="ins", name="psqk",
                               padded_shape=[128, 512], bufs=2)
                x_all, w_all, dst, boff, w0off = (
                    (q_all, wq_all, wqT_sb, 0, 3840) if which == "q"
                    else (k_all, wk_all, wkT_sb, 3, 3072)
                )
                for t in range(_KT):
                    wap = (w0_sb[:, w0off + t * 128:w0off + (t + 1) * 128]
                           if m == 0 else w_all[:, m - 1, t, :])
                    if which == "k" and nch == 0:
                        xap = w0_sb[:, t * 512 + c0:t * 512 + c0 + w]
                    elif which == "k":
                        xap = x_all[:, nch - 1, t, c0:c0 + w]
                    else:
                        xap = x_all[:, nch, t, c0:c0 + w]
                    nc.tensor.matmul(
                        ps[:, :], wap, xap,
                        start=(t == 0), stop=(t == _KT - 1),
                    )
                nc.vector.tensor_scalar_add(
                    dst[m][:, nch * 512 + c0:nch * 512 + c0 + w],
                    ps[:, :], b_sb[:, boff + m:boff + m + 1])

            def emit_insert(item):
                # negative offset = appears LATER to the list scheduler, so
                # ready spine work (S/exp/AV) is never displaced by filler
                # whose DMA data may be late on real hardware
                with tc.high_priority(offset=-1000000):
                    if item[0] == "V":
                        emit_v_unit(item[1], item[2], npair=3)
                    else:
                        _, which, m, nch, c0, w = item
                        emit_qk_part(which, m, nch, c0, w)

            # ---- head: pair-0 qchunk-0 q/k projections ----
            # SPINE priority offset: the attention spine (S-pairs, exps, and
            # the head k00/q00 that gate exp#1) is emitted with a large
            # priority offset so the Tile list-scheduler never displaces
            # ready spine work with projection fillers whose DMA data may be
            # late on real hardware.
            with tc.high_priority():
                emit_qk_part("k", 0, 0, 0, 512)
                emit_qk_part("q", 0, 0, 0, 512)

            # ---- attention stream ----
            # deferred normalize state: (av_sb, recip, hA, hB, qsl) of the
            # previous qchunk; its broadcast + muls + out DMAs are emitted a
            # few iterations later so the in-order PE stream never stalls on
            # the reciprocal round-trip at a boundary.
            pending = []
            flush_gate = [None]  # gi at which pending may flush

            def flush_pending(final=False):
                av_sb, recip, fA, fB, fqsl = pending.pop()
                o_sb = att_pool.tile([64, 1024], f32, tag="o", name="o_sb",
                                     bufs=2)
                if not final:
                    # mid-stream: partition-broadcast on GpSimd (idle engine,
                    # zero PE/PSUM cost; latency hidden by the deferral)
                    bc_sb = att_pool.tile([64, 1024], f32, tag="bc",
                                          name="bc_sb", bufs=2)
                    nc.gpsimd.partition_broadcast(bc_sb[0:64, :],
                                                  recip[0:1, :])
                for h, c0 in ((fA, 0), (fB, 512)):
                    csl = slice(c0, c0 + 512)
                    if final:
                        # final tail: K=1 fp32 matmul on the now-idle PE is
                        # faster than waking GpSimd (~3us with its drain)
                        bc_ps = psum.tile([64, 512], f32, tag="ins",
                                          name="bc_ps",
                                          padded_shape=[128, 512], bufs=2)
                        nc.tensor.matmul(bc_ps[0:64, :], ones16[0:1, 0:64],
                                         recip[0:1, csl],
                                         start=True, stop=True)
                        bc_view = bc_ps[0:64, :]
                    else:
                        bc_view = bc_sb[0:64, csl]
                    nc.vector.tensor_mul(
                        o_sb[0:64, csl], av_sb[0:64, csl], bc_view)
                    # tail: scalar queue is idle once inputs are in; split
                    # the final out DMAs across both queues
                    q_eng = nc.scalar if (final and c0 == 0) else nc.sync
                    q_eng.dma_start(
                        outT[h * 64:h * 64 + 64, fqsl], o_sb[0:64, csl])

            # Flat 192-iteration stream: the S/exp pipeline runs a constant
            # 2 iterations ahead ACROSS qchunk and pair boundaries, so the
            # exp stream never rebuilds its lookahead at a boundary. The
            # AV/normalize stream is drained from a gated queue (AV_GATE) so
            # it tolerates DMA/eviction lag without stalling S/exp.
            # p tiles are a manual ring (not a pool): the ring recycle is a
            # plain WAR dep that is implied by each ACT's own S-matmul wait,
            # so no standalone pool-semaphore instruction burns ScalarE time.
            p_ring = [persist.tile([128, 1024], bf16, tag=f"pr{j}",
                                   name=f"pr{j}") for j in range(24)]
            p_tiles = {}

            def emit_s_exp(gi):
                if gi < 16:
                    with tc.high_priority():
                        _emit_s_exp(gi)
                    return
                _emit_s_exp(gi)

            def _emit_s_exp(gi):
                sp, r = divmod(gi, 64)
                sqch, skt = divmod(r, 16)
                qsl = slice(sqch * 512, (sqch + 1) * 512)
                ksl = slice(skt * 128, (skt + 1) * 128)
                s_AB = psum.tile([128, 1024], f32, tag="s",
                                 name="sAB", bufs=2)
                nc.tensor.matmul(
                    s_AB[:, 0:512],
                    wkT_sb[sp][0:64, ksl], wqT_sb[sp][0:64, qsl],
                    start=True, stop=True,
                    tile_position=(0, 0) if _TILEPOS else None,
                )
                nc.tensor.matmul(
                    s_AB[:, 512:1024],
                    wkT_sb[sp][64:128, ksl], wqT_sb[sp][64:128, qsl],
                    start=True, stop=True,
                    tile_position=(64, 0) if _TILEPOS else None,
                )
                p_AB = p_ring[gi % 24]
                nc.scalar.activation(p_AB[:, :], s_AB[:, :], AF.Exp)
                p_tiles[gi] = p_AB

            # per-qchunk AV accumulator state
            av_state = {"avA": None, "avB": None}
            next_av = [0]  # next global AV index to emit

            def emit_av(k):
                qi, kt = divmod(k, _ST)
                p, qch = divmod(qi, _QC)
                hA, hB = 2 * p, 2 * p + 1
                if kt == 0:
                    # two 1-bank accumulators: the eviction copy of half A
                    # can overlap AV15 of half B, and next qchunk's AV0.A
                    # only waits on copyA - shrinks the boundary stall
                    av_state["avA"] = psum.tile(
                        [65, 512], f32, tag="avA", name="avA",
                        padded_shape=[128, 512], bufs=1)
                    av_state["avB"] = psum.tile(
                        [65, 512], f32, tag="avB", name="avB",
                        padded_shape=[128, 512], bufs=1)
                avA, avB = av_state["avA"], av_state["avB"]
                pv = p_tiles.pop(k)
                nc.tensor.matmul(
                    avA[0:65, :], wv_sb[:, kt, hA, 0:65], pv[:, 0:512],
                    start=(kt == 0), stop=(kt == _ST - 1),
                )
                nc.tensor.matmul(
                    avB[0:65, :], wv_sb[:, kt, hB, 0:65], pv[:, 512:1024],
                    start=(kt == 0), stop=(kt == _ST - 1),
                )
                if kt != _ST - 1:
                    return
                # ---- qchunk done: evict + reciprocal (latency-tolerant) ----
                qsl = slice(qch * 512, (qch + 1) * 512)
                av_sb = att_pool.tile([65, 1024], f32, tag="av_sb",
                                      name="av_sb", bufs=2)
                last = k == N_GI - 1
                with tc.high_priority():
                    if last:
                        # final tail: sums rows only; the big value copies
                        # are emitted AFTER the reciprocal bounce below so
                        # the DVE runs the recips as soon as the bounce DMA
                        # lands instead of behind the copies
                        nc.vector.tensor_copy(av_sb[64:65, 512:1024],
                                              avB[64:65, :])
                        nc.vector.tensor_copy(av_sb[64:65, 0:512],
                                              avA[64:65, :])
                    else:
                        # mid-stream: A first - next qchunk's AV0.A is the
                        # head of the PE stream and waits only on copyA
                        nc.vector.tensor_copy(av_sb[0:65, 0:512],
                                              avA[0:65, :])
                        nc.vector.tensor_copy(av_sb[0:65, 512:1024],
                                              avB[0:65, :])
                # reciprocal is partition-serial on DVE; bounce the sums
                # through [128,4] so all lanes work, then bounce back.
                rp = att_pool.tile([128, 8], f32, tag="rp", name="rp",
                                   bufs=2)
                if last:
                    # bf16 recip: the final-flush broadcast matmul runs
                    # single-pass bf16 instead of two-pass fp32 (~1.5us off
                    # the serial tail; ~0.4% on the denominator is inside
                    # the accuracy budget)
                    rp2 = att_pool.tile([128, 8], bf16, tag="rp2f",
                                        name="rp2f", bufs=1)
                    recip = att_pool.tile([1, 1024], bf16, tag="recipf",
                                          name="recipf", bufs=1)
                else:
                    rp2 = att_pool.tile([128, 8], f32, tag="rp2", name="rp2",
                                        bufs=2)
                    recip = att_pool.tile([1, 1024], f32, tag="recip",
                                          name="recip", bufs=2)
                bq_eng = nc.scalar if last else nc.sync
                for c0 in (0, 512):
                    csl = slice(c0, c0 + 512)
                    rsl = slice(c0 // 128, c0 // 128 + 4)
                    bq_eng.dma_start(rp[0:128, rsl], av_sb[64:65, csl])
                    with nc.allow_low_precision(reason="bf16 recip bcast"):
                        nc.vector.reciprocal(rp2[0:128, rsl], rp[0:128, rsl])
                    bq_eng.dma_start(recip[0:1, csl], rp2[0:128, rsl])
                if last:
                    # deferred big value copies (only the muls need them)
                    nc.vector.tensor_copy(av_sb[0:64, 512:1024], avB[0:64, :])
                    nc.vector.tensor_copy(av_sb[0:64, 0:512], avA[0:64, :])
                pending.append((av_sb, recip, hA, hB, qsl))

            emit_s_exp(0)
            emit_s_exp(1)
            for gi in range(N_GI):
                if gi + 2 < N_GI:
                    emit_s_exp(gi + 2)
                if pending and flush_gate[0] is not None and gi >= flush_gate[0]:
                    flush_pending()
                    flush_gate[0] = None
                for item in INSERTS.get(gi, ()):
                    emit_insert(item)
                ndr = 0
                while next_av[0] < N_GI and next_av[0] <= gi \
                        and AV_GATE[next_av[0]] <= gi and ndr < 3:
                    k = next_av[0]
                    emit_av(k)
                    if k % _ST == _ST - 1:
                        flush_gate[0] = gi + 3
                    next_av[0] += 1
                    ndr += 1

            # drain any AVs still gated past the loop end
            while next_av[0] < N_GI:
                if pending and len(pending) > 1:
                    flush_pending()
                emit_av(next_av[0])
                next_av[0] += 1
            while len(pending) > 1:
                flush_pending()
            with tc.high_priority():
                flush_pending(final=True)

    nc.compile()
    return nc


def _get_compiled():
    global _compiled
    if _compiled is None:
        _compiled = _build()
    return _compiled


def make_in_maps(q, k, v, Wq, bq, Wk, bk, Wv, bv):
    import ml_dtypes

    bf16 = ml_dtypes.bfloat16

    def pack_x(xT):  # [768, 2048] -> [128, 4nch, 6kt, 512]
        return np.ascontiguousarray(
            xT.reshape(_KT, 128, 4, 512).transpose(1, 2, 0, 3)).astype(bf16)

    def pack_w(WT):  # [768, 384] -> [128, 3m, 6kt, 128]
        return np.ascontiguousarray(
            WT.reshape(_KT, 128, _NP, 128).transpose(1, 2, 0, 3)).astype(bf16)

    in_maps = []
    for c in range(_NCORES):
        b, g = c // 2, c % 2
        gsl = slice(g * _GSZ, (g + 1) * _GSZ)
        wq_p = pack_w(np.asarray(Wq)[gsl, :].T)
        wk_p = pack_w(np.asarray(Wk)[gsl, :].T)
        k_p = pack_x(np.asarray(k)[b].T)
        # one ramp-critical transfer: [k chunk 0 | Wk0 | Wq0 | bqT | bkT | bv]
        w0b = np.concatenate([
            k_p[:, 0].reshape(128, _KT * 512),
            wk_p[:, 0].reshape(128, _KT * 128),
            wq_p[:, 0].reshape(128, _KT * 128),
            np.asarray(bq)[gsl].reshape(3, 128).T.astype(bf16),
            np.asarray(bk)[gsl].reshape(3, 128).T.astype(bf16),
            np.tile(np.asarray(bv)[gsl][None, :].astype(bf16), (128, 1)),
        ], axis=1)
        in_maps.append({
            "q_pk": pack_x(np.asarray(q)[b].T),
            "k_pk": k_p,
            "v_pk": pack_x(np.asarray(v)[b].T),
            "Wq_pk": wq_p,
            "Wk_pk": wk_p,
            "Wv_pk": np.ascontiguousarray(np.asarray(Wv)[gsl, :].T.reshape(
                _KT, 128, _GSZ).transpose(1, 0, 2)).astype(bf16),
            "W0b_pk": np.ascontiguousarray(w0b),
        })
    return in_maps


def assemble_out(results):
    out = np.zeros((_BS, _SEQ, _DM), np.float32)
    for c in range(_NCORES):
        b, g = c // 2, c % 2
        out[b, :, g * _GSZ:(g + 1) * _GSZ] = np.asarray(
            results[c]["outT"], np.float32
        ).T
    return out


def kernel(q, k, v, Wq, bq, Wk, bk, Wv, bv):
    from concourse.bass_utils import run_bass_kernel_spmd

    nc = _get_compiled()
    in_maps = make_in_maps(q, k, v, Wq, bq, Wk, bk, Wv, bv)
    res = run_bass_kernel_spmd(nc, in_maps, core_ids=list(range(_NCORES)))
    return assemble_out(res.results)
